# revision 1
# baseline (speedup 1.0000x reference)
"""BiMamba Trainium2 kernel.

Sharding: 8 cores = (direction f/r) x (batch 2) x (d_inner half 2), SPMD
(one program, per-core data).  The host permutes channel order so each
core's own 512 scan channels occupy positions 0..511; xi/conv are computed
for all 1024 channels on every core (x_proj needs the full d_inner
contraction) with the other half's x_proj contribution accumulated into
PSUM on the fly; z/dt/scan/out_proj cover only the own half.  Partial
out_proj results are summed on the host; the reverse direction is flipped
on the host.

Device pipeline (feature-major [feature, token] layouts, f32r matmuls):
  A) in_proj -> xi; depthwise conv as 4 accumulated diag matmuls;
     silu via sigmoid*x on ScalarE+VectorE; x_proj accumulated over all 8
     channel tiles; z -> silu -> gT
  B) x_proj psum -> dt_raw (f32r) and B/C rows (bf16); dt_proj ->
     softplus(ln(1+exp)) -> dtT (bf16); u = dt*xc (bf16)
  C) selective scan, per (pt pair, state s): broadcast B_s/C_s rows to 128
     partitions via partition-step-0 DMA (bf16); per channel tile:
     dA = exp(A_s*dt) on ScalarE (f32r), dBu = u*B_bc (bf16 2x),
     full-length tensor_tensor_scan on VectorE (fp32 state), hc = h*C_bc
     (bf16 2x), and accumulate y = D*xc + sum_s hc via identity/diag
     matmuls into PSUM (TensorE does the adds)
  D) y_gated = y_psum * silu(z) -> f32r; out_proj partial -> DRAM from PSUM
"""
import os
from contextlib import ExitStack

import numpy as np

import concourse.bacc as bacc
import concourse.bass as bass
import concourse.tile as tile
from concourse import mybir
from concourse.bass_utils import run_bass_kernel_spmd

F32 = mybir.dt.float32
BF16 = mybir.dt.bfloat16
F32R = mybir.dt.float32r
AF = mybir.ActivationFunctionType
OP = mybir.AluOpType
NPBF16 = mybir.dt.np(mybir.dt.bfloat16)

DIM = 512
D_STATE = 16
D_CONV = 4
D_INNER = 1024
DT_RANK = 32
B_SZ = 2
SEQ = 2048
HALF = 512
NPT = HALF // 128     # 4 own-channel partition tiles
NFT = D_INNER // 128  # 8 full-channel partition tiles
NC_ = SEQ // 512      # 4 token chunks
NXD = DT_RANK + 2 * D_STATE  # 64

_PROG_CACHE = {}


def _build_program():
    if "nc" in _PROG_CACHE:
        return _PROG_CACHE["nc"]
    nc = bacc.Bacc("TRN2", target_bir_lowering=False, debug=False)

    xT = nc.dram_tensor("xT", [128, 4, SEQ], F32R, kind="ExternalInput")
    w_in = nc.dram_tensor("w_in", [128, 4, D_INNER + HALF], F32R, kind="ExternalInput")
    convw = nc.dram_tensor("convw", [128, NFT, D_CONV], F32, kind="ExternalInput")
    identr = nc.dram_tensor("identr", [128, 128], F32R, kind="ExternalInput")
    convb = nc.dram_tensor("convb", [128, NFT, 1], F32, kind="ExternalInput")
    w_xp = nc.dram_tensor("w_xp", [128, NFT, NXD], F32R, kind="ExternalInput")
    w_dt = nc.dram_tensor("w_dt", [DT_RANK, HALF], F32R, kind="ExternalInput")
    dtb = nc.dram_tensor("dtb", [128, NPT, 1], F32, kind="ExternalInput")
    Acol = nc.dram_tensor("Acol", [128, NPT, D_STATE], F32, kind="ExternalInput")
    diagD = nc.dram_tensor("diagD", [128, NPT, 128], F32R, kind="ExternalInput")
    ident = nc.dram_tensor("ident", [128, 128], BF16, kind="ExternalInput")
    w_out = nc.dram_tensor("w_out", [128, NPT, DIM], F32R, kind="ExternalInput")
    zero3 = nc.dram_tensor("zero3", [128, 3], F32R, kind="ExternalInput")
    oT = nc.dram_tensor("oT", [128, 4, SEQ], F32, kind="ExternalOutput")

    loop_n = int(os.environ.get("BIMAMBA_LOOP", "0"))
    with tile.TileContext(nc) as tc, ExitStack() as est:
        if loop_n > 1:
            est.enter_context(tc.For_i(0, loop_n, 1))
        pP = est.enter_context(tc.tile_pool(name="pP", bufs=1))
        pDram = est.enter_context(tc.tile_pool(name="pDram", bufs=1, space="DRAM"))
        bcd = pDram.tile([2 * D_STATE, SEQ], BF16)

        gT = pP.tile([128, NPT, SEQ], F32)        # silu(z), own half
        xc_own = pP.tile([128, NPT, SEQ], F32R)   # silu(conv(xi)), own half
        dbc_raw = pP.tile([DT_RANK, SEQ], F32R)   # dt_raw rows
        bcb = pP.tile([2 * D_STATE, SEQ], BF16)   # rows 0..15 = B, 16..31 = C

        with tc.tile_pool(name="psX", bufs=4, space="PSUM") as psX:
            psx = []
            for _c in range(NC_):
                psx_t = psX.tile([NXD, 512], F32, tag="xp")
                psx.append(psx_t)

            # ---------- Phase A ----------
            with tc.tile_pool(name="pA", bufs=1) as pA, \
                 tc.tile_pool(name="pAw", bufs=2) as pAw, \
                 tc.tile_pool(name="pXi", bufs=2) as pXi, \
                 tc.tile_pool(name="psA", bufs=3, space="PSUM") as psA:
                sb_xT = pA.tile([128, 4, SEQ], F32R)
                nc.sync.dma_start(sb_xT[:], xT[:])
                sb_cb = pA.tile([128, NFT, 1], F32)
                sb_wxp = pA.tile([128, NFT, NXD], F32R)
                sb_cw = pA.tile([128, NFT, D_CONV], F32)
                sb_idr = pA.tile([128, 128], F32R)
                nc.sync.dma_start(sb_cb[:], convb[:])
                nc.sync.dma_start(sb_wxp[:], w_xp[:])
                nc.sync.dma_start(sb_cw[:], convw[:])
                nc.sync.dma_start(sb_idr[:], identr[:])

                # xi/conv channel tiles first (x_proj finishes earlier so the
                # scan phase can start); z tiles last
                for m in list(range(8)) + list(range(8, 12)):
                    win_m = pAw.tile([128, 4, 128], F32R, tag="win")
                    nc.sync.dma_start(win_m[:], w_in[:, :, m * 128:(m + 1) * 128])
                    xi_pad = None
                    if m < 8:
                        xi_pad = pXi.tile([128, 3 + SEQ], F32R, tag="xi_pad")
                        nc.sync.dma_start(xi_pad[:, 0:3], zero3[:])
                    for c in range(NC_):
                        ps = psA.tile([128, 512], F32, tag="mm")
                        for k in range(4):
                            nc.tensor.matmul(
                                ps[:], win_m[:, k, :],
                                sb_xT[:, k, c * 512:(c + 1) * 512],
                                start=(k == 0), stop=(k == 3))
                        if m < 8:
                            nc.vector.tensor_copy(
                                xi_pad[:, 3 + c * 512: 3 + (c + 1) * 512], ps[:])
                        else:
                            # silu(z) = z * sigmoid(z)
                            sgz = pXi.tile([128, 512], F32, tag="sgz")
                            nc.scalar.activation(sgz[:], ps[:], AF.Sigmoid)
                            nc.vector.tensor_mul(
                                gT[:, m - 8, c * 512:(c + 1) * 512], ps[:], sgz[:])
                    if m < 8:
                        # build diag(conv_w[:, k]) on device: ident * scalar
                        cd_m = pAw.tile([128, D_CONV, 128], F32R, tag="cd")
                        for k in range(D_CONV):
                            nc.vector.tensor_scalar_mul(
                                cd_m[:, k, :], sb_idr[:], sb_cw[:, m, k:k + 1])
                        for c in range(NC_):
                            ps2 = psA.tile([128, 512], F32, tag="mm")
                            for k in range(D_CONV):
                                nc.tensor.matmul(
                                    ps2[:], cd_m[:, k, :],
                                    xi_pad[:, c * 512 + k: c * 512 + k + 512],
                                    start=(k == 0), stop=(k == D_CONV - 1))
                            if m < NPT:
                                xco = xc_own[:, m, c * 512:(c + 1) * 512]
                            else:
                                xco_t = pXi.tile([128, 512], F32R, tag="xco")
                                xco = xco_t[:]
                            # silu(v) = sigmoid(v) * v, v = conv psum + bias
                            sgc = pXi.tile([128, 512], F32, tag="sgc")
                            nc.scalar.activation(sgc[:], ps2[:], AF.Sigmoid,
                                                 bias=sb_cb[:, m, :])
                            nc.vector.scalar_tensor_tensor(
                                xco, ps2[:], sb_cb[:, m, :], sgc[:],
                                OP.add, OP.mult)
                            # accumulate x_proj contribution of this tile
                            nc.tensor.matmul(
                                psx[c][:], sb_wxp[:, m, :], xco,
                                start=(m == 0), stop=(m == 7))

            # unload x_proj accumulators (still inside psX scope)
            for c in range(NC_):
                nc.vector.tensor_copy(dbc_raw[:, c * 512:(c + 1) * 512],
                                      psx[c][0:DT_RANK, :])
                nc.vector.tensor_copy(bcb[:, c * 512:(c + 1) * 512],
                                      psx[c][DT_RANK:NXD, :])
        # stage B/C rows in DRAM so the per-s broadcast DMA can use a
        # partition-step-0 source (SBUF sources reject it)
        nc.sync.dma_start(bcd[:], bcb[:])

        # ---------- Phase B ----------
        pBCD = est.enter_context(tc.tile_pool(name="pBCD", bufs=1))
        dtT = pBCD.tile([128, NPT, SEQ], BF16)
        uT = pBCD.tile([128, NPT, SEQ], BF16)
        sb_A = pBCD.tile([128, NPT, D_STATE], F32)
        sb_dD = pBCD.tile([128, NPT, 128], F32R)
        sb_id = pBCD.tile([128, 128], BF16)
        y_g = pBCD.tile([128, NPT, SEQ], F32R)
        nc.sync.dma_start(sb_A[:], Acol[:])
        nc.sync.dma_start(sb_dD[:], diagD[:])
        nc.sync.dma_start(sb_id[:], ident[:])

        with tc.tile_pool(name="pB", bufs=1) as pB, \
             tc.tile_pool(name="pBt", bufs=2) as pBt, \
             tc.tile_pool(name="psB", bufs=2, space="PSUM") as psB:
            sb_wdt = pB.tile([DT_RANK, HALF], F32R)
            sb_dtb = pB.tile([128, NPT, 1], F32)
            nc.sync.dma_start(sb_wdt[:], w_dt[:])
            nc.sync.dma_start(sb_dtb[:], dtb[:])
            for mt in range(NPT):
                for c in range(NC_):
                    ps3 = psB.tile([128, 512], F32, tag="mm")
                    nc.tensor.matmul(
                        ps3[:], sb_wdt[:, mt * 128:(mt + 1) * 128],
                        dbc_raw[:, c * 512:(c + 1) * 512], start=True, stop=True)
                    # softplus(w) = ln(1 + exp(w)); w = psum + dt_bias
                    spe = pBt.tile([128, 512], F32, tag="spe")
                    nc.scalar.activation(spe[:], ps3[:], AF.Exp, bias=sb_dtb[:, mt, :])
                    nc.scalar.activation(
                        dtT[:, mt, c * 512:(c + 1) * 512], spe[:], AF.Ln, bias=1.0)

            for pt in range(NPT):
                nc.vector.tensor_mul(uT[:, pt, :], dtT[:, pt, :],
                                     xc_own[:, pt, :].bitcast(F32))

        # ---------- Phase C: selective scan ----------
        with tc.tile_pool(name="pC", bufs=2) as pC, \
             tc.tile_pool(name="psC", bufs=8, space="PSUM") as psC:
            for pair in range(2):
                pts = (2 * pair, 2 * pair + 1)
                # y accumulators: one PSUM bank per (pt-in-pair, token chunk)
                yps = {}
                for ptl, pt in enumerate(pts):
                    for q in range(NC_):
                        yps_t = psC.tile([128, 512], F32, tag="yps")
                        yps[(ptl, q)] = yps_t
                        # initialize with D * xc via diag matmul
                        nc.tensor.matmul(
                            yps_t[:], sb_dD[:, pt, :],
                            xc_own[:, pt, q * 512:(q + 1) * 512],
                            start=True, stop=False, skip_group_check=True)
                for s in range(D_STATE):
                    B_bc = pC.tile([128, SEQ], BF16, tag="bbc")
                    C_bc = pC.tile([128, SEQ], BF16, tag="cbc")
                    # split each broadcast into chunk DMAs so they spread
                    # across multiple DMA queues/engines
                    nsp = int(os.environ.get("BIMAMBA_BCSPLIT", "2"))
                    csz = SEQ // nsp
                    for j in range(nsp):
                        brow = bcd[s:s + 1, j * csz:(j + 1) * csz]
                        crow = bcd[D_STATE + s:D_STATE + s + 1, j * csz:(j + 1) * csz]
                        nc.gpsimd.dma_start(B_bc[:, j * csz:(j + 1) * csz], bass.AP(
                            tensor=brow.tensor, offset=brow.offset,
                            ap=[[0, 128]] + list(brow.ap[1:])))
                        nc.gpsimd.dma_start(C_bc[:, j * csz:(j + 1) * csz], bass.AP(
                            tensor=crow.tensor, offset=crow.offset,
                            ap=[[0, 128]] + list(crow.ap[1:])))
                    # high-s multiplies run on GpSimd to offload VectorE
                    # (the scan opcode itself is VectorE-only)
                    gps_thresh = int(os.environ.get("BIMAMBA_GPS", "16"))
                    eng = nc.gpsimd if s >= gps_thresh else nc.vector
                    da_dt = BF16 if os.environ.get("BIMAMBA_DABF16") else F32R
                    for ptl, pt in enumerate(pts):
                        dA = pC.tile([128, SEQ], da_dt, tag="dA")
                        nc.scalar.activation(dA[:], dtT[:, pt, :], AF.Exp,
                                             scale=sb_A[:, pt, s:s + 1])
                        dBu = pC.tile([128, SEQ], BF16, tag="dBu")
                        eng.tensor_mul(dBu[:], uT[:, pt, :], B_bc[:])
                        h = pC.tile([128, SEQ], BF16, tag="h")
                        nc.vector.tensor_tensor_scan(h[:], dA[:], dBu[:], 0.0,
                                                     OP.mult, OP.add)
                        hc = pC.tile([128, SEQ], BF16, tag="hc")
                        eng.tensor_mul(hc[:], h[:], C_bc[:])
                        for q in range(NC_):
                            nc.tensor.matmul(
                                yps[(ptl, q)][:], sb_id[:],
                                hc[:, q * 512:(q + 1) * 512],
                                start=False, stop=(s == D_STATE - 1),
                                skip_group_check=True)
                # gate: y_g = y * silu(z)
                for ptl, pt in enumerate(pts):
                    for q in range(NC_):
                        nc.vector.tensor_mul(
                            y_g[:, pt, q * 512:(q + 1) * 512],
                            yps[(ptl, q)][:],
                            gT[:, pt, q * 512:(q + 1) * 512])

        # ---------- Phase D: out_proj ----------
        with tc.tile_pool(name="pD", bufs=1) as pD, \
             tc.tile_pool(name="pDo", bufs=3) as pDo, \
             tc.tile_pool(name="psD", bufs=3, space="PSUM") as psD:
            sb_wout = pD.tile([128, NPT, DIM], F32R)
            nc.sync.dma_start(sb_wout[:], w_out[:])
            for mt in range(NPT):
                for c in range(NC_):
                    ps4 = psD.tile([128, 512], F32, tag="mm")
                    for k in range(NPT):
                        nc.tensor.matmul(
                            ps4[:], sb_wout[:, k, mt * 128:(mt + 1) * 128],
                            y_g[:, k, c * 512:(c + 1) * 512],
                            start=(k == 0), stop=(k == NPT - 1))
                    ot = pDo.tile([128, 512], F32, tag="ot")
                    nc.scalar.activation(ot[:], ps4[:], AF.Copy)
                    nc.sync.dma_start(oT[:, mt, c * 512:(c + 1) * 512], ot[:])

    nc.compile()
    _PROG_CACHE["nc"] = nc
    return nc


def _prep_core_inputs(x, params, direction, batch, half):
    in_w, conv_w, conv_b, xproj_w, dt_w, dt_b, A_log, D, out_w = params
    xb = x[batch]
    if direction == 1:
        xb = xb[::-1]
    xT = np.ascontiguousarray(xb.T)

    own = np.arange(half * HALF, (half + 1) * HALF)
    other = np.arange((1 - half) * HALF, (2 - half) * HALF)
    perm = np.concatenate([own, other])

    w_in = np.concatenate([in_w[perm], in_w[D_INNER + own]], axis=0).T  # [512, 1536]
    cw = conv_w[perm, 0, :]                                            # [1024, 4]
    convw_ = cw.reshape(NFT, 128, D_CONV).transpose(1, 0, 2)           # [128, NFT, 4]
    ii = np.arange(128)
    convb_ = conv_b[perm].reshape(NFT, 128, 1).transpose(1, 0, 2)
    w_xp = xproj_w[:, perm].T.reshape(NFT, 128, -1).transpose(1, 0, 2)
    w_dt_ = np.ascontiguousarray(dt_w[own].T)
    dtb_ = dt_b[own].reshape(NPT, 128, 1).transpose(1, 0, 2)
    Acol_ = (-np.exp(A_log[own])).reshape(NPT, 128, D_STATE).transpose(1, 0, 2)
    dD = np.zeros((128, NPT, 128), np.float32)
    Dr = D[own].reshape(NPT, 128)
    for ptn in range(NPT):
        dD[ii, ptn, ii] = Dr[ptn]
    w_out_ = out_w[:, own].T.reshape(NPT, 128, DIM).transpose(1, 0, 2)

    def c32(a):
        return np.ascontiguousarray(a, dtype=np.float32)

    return {
        "xT": c32(xT.reshape(4, 128, SEQ).transpose(1, 0, 2)),
        "w_in": c32(w_in.reshape(4, 128, -1).transpose(1, 0, 2)),
        "convw": c32(convw_),
        "identr": np.eye(128, dtype=np.float32),
        "convb": c32(convb_),
        "w_xp": c32(w_xp),
        "w_dt": c32(w_dt_),
        "dtb": c32(dtb_),
        "Acol": c32(Acol_),
        "diagD": c32(dD),
        "ident": np.eye(128, dtype=NPBF16),
        "w_out": c32(w_out_),
        "zero3": np.zeros((128, 3), np.float32),
    }


def _run(nc, in_maps):
    if os.environ.get("BIMAMBA_SIM"):
        from concourse.bass_interp import CoreSim
        results = []
        n = int(os.environ.get("BIMAMBA_SIM_CORES", "8"))
        for m in in_maps[:n]:
            sim = CoreSim(nc)
            for k, v in m.items():
                sim.tensor(k)[:] = v
            sim.simulate()
            results.append({"oT": np.array(sim.tensor("oT"))})
        return results
    return run_bass_kernel_spmd(nc, in_maps, core_ids=list(range(8))).results


def _prep_all(inputs):
    x = np.asarray(inputs["x"], np.float32)
    names = ["in_w", "conv_w", "conv_b", "xproj_w", "dt_w", "dt_b", "A_log", "D", "out_w"]
    fp = tuple(np.asarray(inputs["f_" + n], np.float32) for n in names)
    rp = tuple(np.asarray(inputs["r_" + n], np.float32) for n in names)
    in_maps = []
    meta = []
    for d in (0, 1):
        for b in range(B_SZ):
            for h in (0, 1):
                in_maps.append(_prep_core_inputs(x, fp if d == 0 else rp, d, b, h))
                meta.append((d, b, h))
    return in_maps, meta


def kernel(**inputs):
    nc = _build_program()
    in_maps, meta = _prep_all(inputs)
    results = _run(nc, in_maps)

    acc = np.zeros((2, B_SZ, SEQ, DIM), np.float32)
    for (d, b, h), r in zip(meta, results):
        oTv = r["oT"]
        o = oTv.transpose(1, 0, 2).reshape(DIM, SEQ).T
        if d == 1:
            o = o[::-1]
        acc[d, b] += o
    out = 0.5 * (acc[0] + acc[1])
    return out.astype(np.float32)



# revision 9
# speedup vs baseline: 1.2078x; 1.2078x over previous
"""BiMamba Trainium2 kernel.

Sharding: 8 cores = (direction f/r) x (batch 2) x (d_inner half 2), SPMD
(one program, per-core data).  The host permutes channel order so each
core's own 512 scan channels occupy positions 0..511; xi/conv are computed
for all 1024 channels on every core (x_proj needs the full d_inner
contraction) with the other half's x_proj contribution accumulated into
PSUM on the fly; z/dt/scan/out_proj cover only the own half.  Partial
out_proj results are summed on the host; the reverse direction is flipped
on the host.

Device pipeline (feature-major [feature, token] layouts, f32r matmuls):
  A) in_proj -> xi; depthwise conv as 4 accumulated diag matmuls;
     silu via single AF.Silu activation (bias folds conv_b); x_proj
     accumulated over all 8 channel tiles; z -> AF.Silu -> gT
  B) x_proj psum -> dt_raw (f32r) and B/C rows (bf16); dt_proj ->
     AF.Softplus (bias folds dt_b) -> dtT (bf16); u = dt*xc (bf16)
  C) selective scan, per (pt pair, state s): broadcast B_s/C_s rows to 128
     partitions via partition-step-0 DMA (bf16), queues split SP/Pool;
     per channel tile: dA = exp(A_s*dt) on ScalarE, dBu = u*B_bc (bf16 2x
     DVE), full-length tensor_tensor_scan split across DVE and Pool
     (fp32 state), hc = h*C_bc (bf16 2x), and accumulate
     y = D*xc + sum_s hc via identity/diag matmuls into PSUM
  D) y_gated = y_psum * silu(z) -> f32r; out_proj partial -> DRAM from PSUM
"""
import os
from contextlib import ExitStack

import numpy as np

import concourse.bacc as bacc
import concourse.bass as bass
import concourse.tile as tile
from concourse import mybir
from concourse.bass_utils import run_bass_kernel_spmd

F32 = mybir.dt.float32
BF16 = mybir.dt.bfloat16
F32R = mybir.dt.float32r
AF = mybir.ActivationFunctionType
OP = mybir.AluOpType
NPBF16 = mybir.dt.np(mybir.dt.bfloat16)

DIM = 512
D_STATE = 16
D_CONV = 4
D_INNER = 1024
DT_RANK = 32
B_SZ = 2
SEQ = 2048
HALF = 512
NPT = HALF // 128     # 4 own-channel partition tiles
NFT = D_INNER // 128  # 8 full-channel partition tiles
NC_ = SEQ // 512      # 4 token chunks
NXD = DT_RANK + 2 * D_STATE  # 64

_PROG_CACHE = {}


def _build_program():
    if "nc" in _PROG_CACHE:
        return _PROG_CACHE["nc"]
    nc = bacc.Bacc("TRN2", target_bir_lowering=False, debug=False)

    xT = nc.dram_tensor("xT", [128, 4, SEQ], F32R, kind="ExternalInput")
    w_in = nc.dram_tensor("w_in", [128, 4, D_INNER + HALF], F32R, kind="ExternalInput")
    convw = nc.dram_tensor("convw", [128, NFT, D_CONV], F32, kind="ExternalInput")
    identr = nc.dram_tensor("identr", [128, 128], F32R, kind="ExternalInput")
    convb = nc.dram_tensor("convb", [128, NFT, 1], F32, kind="ExternalInput")
    w_xp = nc.dram_tensor("w_xp", [128, NFT, NXD], F32R, kind="ExternalInput")
    w_dt = nc.dram_tensor("w_dt", [DT_RANK, HALF], F32R, kind="ExternalInput")
    dtb = nc.dram_tensor("dtb", [128, NPT, 1], F32, kind="ExternalInput")
    Acol = nc.dram_tensor("Acol", [128, NPT, D_STATE], F32, kind="ExternalInput")
    diagD = nc.dram_tensor("diagD", [128, NPT, 128], F32R, kind="ExternalInput")
    ident = nc.dram_tensor("ident", [128, 128], BF16, kind="ExternalInput")
    w_out = nc.dram_tensor("w_out", [128, NPT, DIM], F32R, kind="ExternalInput")
    zero3 = nc.dram_tensor("zero3", [128, 3], F32R, kind="ExternalInput")
    oT = nc.dram_tensor("oT", [128, 4, SEQ], F32, kind="ExternalOutput")

    loop_n = int(os.environ.get("BIMAMBA_LOOP", "0"))
    # tensor_tensor_scan is DVE-only on HW (Pool ISA check rejects it);
    # threshold kept as an env knob but defaults to all-DVE
    scan_pool_t = int(os.environ.get("BIMAMBA_SCANPOOL_T", "16"))
    with tile.TileContext(nc) as tc, ExitStack() as est:
        if loop_n > 1:
            est.enter_context(tc.For_i(0, loop_n, 1))
        pP = est.enter_context(tc.tile_pool(name="pP", bufs=1))
        pDram = est.enter_context(tc.tile_pool(name="pDram", bufs=1, space="DRAM"))
        bcd = pDram.tile([2 * D_STATE, SEQ], BF16)

        gT = pP.tile([128, NPT, SEQ], F32)        # silu(z), own half
        xc_own = pP.tile([128, NPT, SEQ], F32R)   # silu(conv(xi)), own half
        dbc_raw = pP.tile([DT_RANK, SEQ], F32R)   # dt_raw rows
        bcb = pP.tile([2 * D_STATE, SEQ], BF16)   # rows 0..15 = B, 16..31 = C

        with tc.tile_pool(name="psX", bufs=4, space="PSUM") as psX:
            psx = []
            for _c in range(NC_):
                psx_t = psX.tile([NXD, 512], F32, tag="xp")
                psx.append(psx_t)

            # ---------- Phase A ----------
            with tc.tile_pool(name="pA", bufs=1) as pA, \
                 tc.tile_pool(name="pAw", bufs=2) as pAw, \
                 tc.tile_pool(name="pXi", bufs=2) as pXi, \
                 tc.tile_pool(name="psA", bufs=3, space="PSUM") as psA:
                sb_xT = pA.tile([128, 4, SEQ], F32R)
                nc.sync.dma_start(sb_xT[:], xT[:])
                sb_cb = pA.tile([128, NFT, 1], F32)
                sb_wxp = pA.tile([128, NFT, NXD], F32R)
                sb_cw = pA.tile([128, NFT, D_CONV], F32)
                sb_idr = pA.tile([128, 128], F32R)
                nc.sync.dma_start(sb_cb[:], convb[:])
                nc.sync.dma_start(sb_wxp[:], w_xp[:])
                nc.sync.dma_start(sb_cw[:], convw[:])
                nc.sync.dma_start(sb_idr[:], identr[:])

                # xi/conv channel tiles first (x_proj finishes earlier so the
                # scan phase can start); z tiles last
                for m in list(range(8)) + list(range(8, 12)):
                    win_m = pAw.tile([128, 4, 128], F32R, tag="win")
                    nc.sync.dma_start(win_m[:], w_in[:, :, m * 128:(m + 1) * 128])
                    xi_pad = None
                    if m < 8:
                        xi_pad = pXi.tile([128, 3 + SEQ], F32R, tag="xi_pad")
                        nc.sync.dma_start(xi_pad[:, 0:3], zero3[:])
                    for c in range(NC_):
                        ps = psA.tile([128, 512], F32, tag="mm")
                        for k in range(4):
                            nc.tensor.matmul(
                                ps[:], win_m[:, k, :],
                                sb_xT[:, k, c * 512:(c + 1) * 512],
                                start=(k == 0), stop=(k == 3))
                        if m < 8:
                            # PSUM -> SBUF copy on ScalarE (frees DVE)
                            nc.scalar.activation(
                                xi_pad[:, 3 + c * 512: 3 + (c + 1) * 512],
                                ps[:], AF.Copy)
                        else:
                            # silu(z) in one activation op
                            nc.scalar.activation(
                                gT[:, m - 8, c * 512:(c + 1) * 512], ps[:],
                                AF.Silu)
                    if m < 8:
                        # build diag(conv_w[:, k]) on device: ident * scalar
                        cd_m = pAw.tile([128, D_CONV, 128], F32R, tag="cd")
                        for k in range(D_CONV):
                            nc.vector.tensor_scalar_mul(
                                cd_m[:, k, :], sb_idr[:], sb_cw[:, m, k:k + 1])
                        for c in range(NC_):
                            ps2 = psA.tile([128, 512], F32, tag="mm")
                            for k in range(D_CONV):
                                nc.tensor.matmul(
                                    ps2[:], cd_m[:, k, :],
                                    xi_pad[:, c * 512 + k: c * 512 + k + 512],
                                    start=(k == 0), stop=(k == D_CONV - 1))
                            if m < NPT:
                                xco = xc_own[:, m, c * 512:(c + 1) * 512]
                            else:
                                xco_t = pXi.tile([128, 512], F32R, tag="xco")
                                xco = xco_t[:]
                            # silu(conv + bias) in one activation op
                            nc.scalar.activation(xco, ps2[:], AF.Silu,
                                                 bias=sb_cb[:, m, :])
                            # accumulate x_proj contribution of this tile
                            nc.tensor.matmul(
                                psx[c][:], sb_wxp[:, m, :], xco,
                                start=(m == 0), stop=(m == 7))

            # unload x_proj accumulators (still inside psX scope)
            for c in range(NC_):
                nc.scalar.activation(dbc_raw[:, c * 512:(c + 1) * 512],
                                     psx[c][0:DT_RANK, :], AF.Copy)
                nc.scalar.activation(bcb[:, c * 512:(c + 1) * 512],
                                     psx[c][DT_RANK:NXD, :], AF.Copy)
        # stage B/C rows in DRAM so the per-s broadcast DMA can use a
        # partition-step-0 source (SBUF sources reject it)
        nc.sync.dma_start(bcd[:], bcb[:])

        # ---------- Phase B ----------
        pBCD = est.enter_context(tc.tile_pool(name="pBCD", bufs=1))
        dtT = pBCD.tile([128, NPT, SEQ], BF16)
        uT = pBCD.tile([128, NPT, SEQ], BF16)
        sb_A = pBCD.tile([128, NPT, D_STATE], F32)
        sb_dD = pBCD.tile([128, NPT, 128], F32R)
        sb_id = pBCD.tile([128, 128], BF16)
        y_g = pBCD.tile([128, NPT, SEQ], F32R)
        nc.sync.dma_start(sb_A[:], Acol[:])
        nc.sync.dma_start(sb_dD[:], diagD[:])
        nc.sync.dma_start(sb_id[:], ident[:])

        with tc.tile_pool(name="pB", bufs=1) as pB, \
             tc.tile_pool(name="pBt", bufs=2) as pBt, \
             tc.tile_pool(name="psB", bufs=2, space="PSUM") as psB:
            sb_wdt = pB.tile([DT_RANK, HALF], F32R)
            sb_dtb = pB.tile([128, NPT, 1], F32)
            nc.sync.dma_start(sb_wdt[:], w_dt[:])
            nc.sync.dma_start(sb_dtb[:], dtb[:])
            for mt in range(NPT):
                for c in range(NC_):
                    ps3 = psB.tile([128, 512], F32, tag="mm")
                    nc.tensor.matmul(
                        ps3[:], sb_wdt[:, mt * 128:(mt + 1) * 128],
                        dbc_raw[:, c * 512:(c + 1) * 512], start=True, stop=True)
                    # softplus(w) = ln(1 + exp(w)); w = psum + dt_bias
                    # (no softplus table in the deployed pwp set)
                    spe = pBt.tile([128, 512], F32, tag="spe")
                    nc.scalar.activation(spe[:], ps3[:], AF.Exp, bias=sb_dtb[:, mt, :])
                    nc.scalar.activation(
                        dtT[:, mt, c * 512:(c + 1) * 512], spe[:], AF.Ln, bias=1.0)

            for pt in range(NPT):
                nc.vector.tensor_mul(uT[:, pt, :], dtT[:, pt, :],
                                     xc_own[:, pt, :].bitcast(F32))

        # ---------- Phase C: selective scan ----------
        bc_bufs = int(os.environ.get("BIMAMBA_BCBUFS", "3"))
        pc_bufs = int(os.environ.get("BIMAMBA_PCBUFS", "2"))
        with tc.tile_pool(name="pBc", bufs=bc_bufs) as pBc, \
             tc.tile_pool(name="pC", bufs=pc_bufs) as pC, \
             tc.tile_pool(name="psC", bufs=8, space="PSUM") as psC:
            for pair in range(2):
                pts = (2 * pair, 2 * pair + 1)
                # y accumulators: one PSUM bank per (pt-in-pair, token chunk)
                yps = {}
                for ptl, pt in enumerate(pts):
                    for q in range(NC_):
                        yps_t = psC.tile([128, 512], F32, tag="yps")
                        yps[(ptl, q)] = yps_t
                        # initialize with D * xc via diag matmul
                        nc.tensor.matmul(
                            yps_t[:], sb_dD[:, pt, :],
                            xc_own[:, pt, q * 512:(q + 1) * 512],
                            start=True, stop=False, skip_group_check=True)
                for s in range(D_STATE):
                    B_bc = pBc.tile([128, SEQ], BF16, tag="bbc")
                    C_bc = pBc.tile([128, SEQ], BF16, tag="cbc")
                    # split each broadcast into chunk DMAs; alternate between
                    # the SP HWDGE queue and the Pool SWDGE queue (by state
                    # parity) so neither sequencer serializes on the transfers
                    nsp = int(os.environ.get("BIMAMBA_BCSPLIT", "1"))
                    csz = SEQ // nsp
                    for j in range(nsp):
                        deng = nc.sync if ((s + j) % 2 == 0) else nc.gpsimd
                        brow = bcd[s:s + 1, j * csz:(j + 1) * csz]
                        crow = bcd[D_STATE + s:D_STATE + s + 1, j * csz:(j + 1) * csz]
                        deng.dma_start(B_bc[:, j * csz:(j + 1) * csz], bass.AP(
                            tensor=brow.tensor, offset=brow.offset,
                            ap=[[0, 128]] + list(brow.ap[1:])))
                        deng.dma_start(C_bc[:, j * csz:(j + 1) * csz], bass.AP(
                            tensor=crow.tensor, offset=crow.offset,
                            ap=[[0, 128]] + list(crow.ap[1:])))
                    for ptl, pt in enumerate(pts):
                        dA = pC.tile([128, SEQ], F32R, tag="dA")
                        nc.scalar.activation(dA[:], dtT[:, pt, :], AF.Exp,
                                             scale=sb_A[:, pt, s:s + 1])
                        dBu = pC.tile([128, SEQ], BF16, tag="dBu")
                        nc.vector.tensor_mul(dBu[:], uT[:, pt, :], B_bc[:])
                        h = pC.tile([128, SEQ], BF16, tag="h")
                        seng = nc.gpsimd if s >= scan_pool_t else nc.vector
                        seng.tensor_tensor_scan(h[:], dA[:], dBu[:], 0.0,
                                                OP.mult, OP.add)
                        hc = pC.tile([128, SEQ], BF16, tag="hc")
                        nc.vector.tensor_mul(hc[:], h[:], C_bc[:])
                        for q in range(NC_):
                            nc.tensor.matmul(
                                yps[(ptl, q)][:], sb_id[:],
                                hc[:, q * 512:(q + 1) * 512],
                                start=False, stop=(s == D_STATE - 1),
                                skip_group_check=True)
                # gate: y_g = y * silu(z)
                for ptl, pt in enumerate(pts):
                    for q in range(NC_):
                        nc.vector.tensor_mul(
                            y_g[:, pt, q * 512:(q + 1) * 512],
                            yps[(ptl, q)][:],
                            gT[:, pt, q * 512:(q + 1) * 512])

        # ---------- Phase D: out_proj ----------
        with tc.tile_pool(name="pD", bufs=1) as pD, \
             tc.tile_pool(name="pDo", bufs=3) as pDo, \
             tc.tile_pool(name="psD", bufs=3, space="PSUM") as psD:
            sb_wout = pD.tile([128, NPT, DIM], F32R)
            nc.sync.dma_start(sb_wout[:], w_out[:])
            for mt in range(NPT):
                for c in range(NC_):
                    ps4 = psD.tile([128, 512], F32, tag="mm")
                    for k in range(NPT):
                        nc.tensor.matmul(
                            ps4[:], sb_wout[:, k, mt * 128:(mt + 1) * 128],
                            y_g[:, k, c * 512:(c + 1) * 512],
                            start=(k == 0), stop=(k == NPT - 1))
                    ot = pDo.tile([128, 512], F32, tag="ot")
                    nc.scalar.activation(ot[:], ps4[:], AF.Copy)
                    nc.sync.dma_start(oT[:, mt, c * 512:(c + 1) * 512], ot[:])

    nc.compile()
    _PROG_CACHE["nc"] = nc
    return nc


def _prep_core_inputs(x, params, direction, batch, half):
    in_w, conv_w, conv_b, xproj_w, dt_w, dt_b, A_log, D, out_w = params
    xb = x[batch]
    if direction == 1:
        xb = xb[::-1]
    xT = np.ascontiguousarray(xb.T)

    own = np.arange(half * HALF, (half + 1) * HALF)
    other = np.arange((1 - half) * HALF, (2 - half) * HALF)
    perm = np.concatenate([own, other])

    w_in = np.concatenate([in_w[perm], in_w[D_INNER + own]], axis=0).T  # [512, 1536]
    cw = conv_w[perm, 0, :]                                            # [1024, 4]
    convw_ = cw.reshape(NFT, 128, D_CONV).transpose(1, 0, 2)           # [128, NFT, 4]
    ii = np.arange(128)
    convb_ = conv_b[perm].reshape(NFT, 128, 1).transpose(1, 0, 2)
    w_xp = xproj_w[:, perm].T.reshape(NFT, 128, -1).transpose(1, 0, 2)
    w_dt_ = np.ascontiguousarray(dt_w[own].T)
    dtb_ = dt_b[own].reshape(NPT, 128, 1).transpose(1, 0, 2)
    Acol_ = (-np.exp(A_log[own])).reshape(NPT, 128, D_STATE).transpose(1, 0, 2)
    dD = np.zeros((128, NPT, 128), np.float32)
    Dr = D[own].reshape(NPT, 128)
    for ptn in range(NPT):
        dD[ii, ptn, ii] = Dr[ptn]
    w_out_ = out_w[:, own].T.reshape(NPT, 128, DIM).transpose(1, 0, 2)

    def c32(a):
        return np.ascontiguousarray(a, dtype=np.float32)

    return {
        "xT": c32(xT.reshape(4, 128, SEQ).transpose(1, 0, 2)),
        "w_in": c32(w_in.reshape(4, 128, -1).transpose(1, 0, 2)),
        "convw": c32(convw_),
        "identr": np.eye(128, dtype=np.float32),
        "convb": c32(convb_),
        "w_xp": c32(w_xp),
        "w_dt": c32(w_dt_),
        "dtb": c32(dtb_),
        "Acol": c32(Acol_),
        "diagD": c32(dD),
        "ident": np.eye(128, dtype=NPBF16),
        "w_out": c32(w_out_),
        "zero3": np.zeros((128, 3), np.float32),
    }


def _run(nc, in_maps):
    if os.environ.get("BIMAMBA_SIM"):
        from concourse.bass_interp import CoreSim
        results = []
        n = int(os.environ.get("BIMAMBA_SIM_CORES", "8"))
        for m in in_maps[:n]:
            sim = CoreSim(nc)
            for k, v in m.items():
                sim.tensor(k)[:] = v
            sim.simulate()
            results.append({"oT": np.array(sim.tensor("oT"))})
        return results
    return run_bass_kernel_spmd(nc, in_maps, core_ids=list(range(8))).results


def _prep_all(inputs):
    x = np.asarray(inputs["x"], np.float32)
    names = ["in_w", "conv_w", "conv_b", "xproj_w", "dt_w", "dt_b", "A_log", "D", "out_w"]
    fp = tuple(np.asarray(inputs["f_" + n], np.float32) for n in names)
    rp = tuple(np.asarray(inputs["r_" + n], np.float32) for n in names)
    in_maps = []
    meta = []
    for d in (0, 1):
        for b in range(B_SZ):
            for h in (0, 1):
                in_maps.append(_prep_core_inputs(x, fp if d == 0 else rp, d, b, h))
                meta.append((d, b, h))
    return in_maps, meta


def kernel(**inputs):
    nc = _build_program()
    in_maps, meta = _prep_all(inputs)
    results = _run(nc, in_maps)

    acc = np.zeros((2, B_SZ, SEQ, DIM), np.float32)
    for (d, b, h), r in zip(meta, results):
        oTv = r["oT"]
        o = oTv.transpose(1, 0, 2).reshape(DIM, SEQ).T
        if d == 1:
            o = o[::-1]
        acc[d, b] += o
    out = 0.5 * (acc[0] + acc[1])
    return out.astype(np.float32)


# revision 10
# speedup vs baseline: 1.2316x; 1.0197x over previous
"""BiMamba Trainium2 kernel, chunk-pipelined (v3).

Sharding: 8 cores = (direction f/r) x (batch 2) x (d_inner half 2), SPMD.
Host permutes channels so each core's own 512 scan channels are first;
xi/conv/x_proj cover all 1024 channels per core; out_proj partials summed
on host; reverse direction flipped on host.

v3 restructure: the sequence is processed in 4 chunks of 512 tokens and
the whole per-chunk pipeline (in_proj/conv -> x_proj -> dt/B/C -> scan ->
gate -> out_proj) is software-pipelined: chunk c+1's PE/Act work (in_proj,
conv, silu) is emitted woven between chunk c's scan iterations so the
in-order engine queues overlap phases.  Scans are chunked and chained via
a per-(s,pt) boundary-state tile (tensor_tensor_scan initial=).  The
selective-scan elementwise ops run as fused [128,4,512] 3D-AP ops (the
B/C broadcast operand uses a 0-stride middle dim) since DVE is the
bottleneck engine: scans + float muls are DVE-only on TRN2 (Pool's ISA
rejects the scan opcode and float tensor_tensor; Pool cannot touch PSUM).
B_s/C_s rows for one (chunk, s) are fetched by a single partition-step-0
DMA shared by all 4 channel tiles, alternating SP/Pool queues, prefetched
3 states ahead.
"""
import os
from contextlib import ExitStack

import numpy as np

import concourse.bacc as bacc
import concourse.bass as bass
import concourse.tile as tile
from concourse import mybir
from concourse.bass_utils import run_bass_kernel_spmd

F32 = mybir.dt.float32
BF16 = mybir.dt.bfloat16
F32R = mybir.dt.float32r
AF = mybir.ActivationFunctionType
OP = mybir.AluOpType
NPBF16 = mybir.dt.np(mybir.dt.bfloat16)

DIM = 512
D_STATE = 16
D_CONV = 4
D_INNER = 1024
DT_RANK = 32
B_SZ = 2
SEQ = 2048
HALF = 512
NPT = HALF // 128     # 4 own-channel partition tiles
NFT = D_INNER // 128  # 8 full-channel partition tiles
NXD = DT_RANK + 2 * D_STATE  # 64
CH = 512              # token chunk
NCH = SEQ // CH       # 4 chunks

_PROG_CACHE = {}


def _bcast_rows(src_row, n_rows, row_stride, n_part=128):
    """Partition-step-0 AP: every partition reads the same n_rows rows."""
    return bass.AP(
        tensor=src_row.tensor, offset=src_row.offset,
        ap=[[0, n_part], [row_stride, n_rows]] + list(src_row.ap[1:]))


def _mid_bcast(sl2d, n_mid):
    """[128, N] slice -> [128, n_mid, N] AP with 0-stride middle dim."""
    return bass.AP(
        tensor=sl2d.tensor, offset=sl2d.offset,
        ap=[list(sl2d.ap[0]), [0, n_mid], list(sl2d.ap[-1])])


def _build_program():
    if "nc" in _PROG_CACHE:
        return _PROG_CACHE["nc"]
    nc = bacc.Bacc("TRN2", target_bir_lowering=False, debug=False)

    xT = nc.dram_tensor("xT", [128, 4, SEQ], F32R, kind="ExternalInput")
    w_in = nc.dram_tensor("w_in", [128, 4, D_INNER + HALF], F32R, kind="ExternalInput")
    convw = nc.dram_tensor("convw", [128, NFT, D_CONV], F32, kind="ExternalInput")
    convd = nc.dram_tensor("convd", [128, NFT, D_CONV, 128], BF16, kind="ExternalInput")
    convb = nc.dram_tensor("convb", [128, NFT, 1], F32, kind="ExternalInput")
    w_xp = nc.dram_tensor("w_xp", [128, NFT, NXD], BF16, kind="ExternalInput")
    w_dt = nc.dram_tensor("w_dt", [DT_RANK, HALF], F32R, kind="ExternalInput")
    dtb = nc.dram_tensor("dtb", [128, NPT, 1], F32, kind="ExternalInput")
    Acol = nc.dram_tensor("Acol", [128, NPT, D_STATE], F32, kind="ExternalInput")
    diagD = nc.dram_tensor("diagD", [128, NPT, 128], BF16, kind="ExternalInput")
    ident = nc.dram_tensor("ident", [128, 128], BF16, kind="ExternalInput")
    w_out = nc.dram_tensor("w_out", [128, NPT, DIM], F32R, kind="ExternalInput")
    oT = nc.dram_tensor("oT", [128, 4, SEQ], F32, kind="ExternalOutput")

    loop_n = int(os.environ.get("BIMAMBA_LOOP", "0"))
    with tile.TileContext(nc) as tc, ExitStack() as est:
        if loop_n > 1:
            est.enter_context(tc.For_i(0, loop_n, 1))
        pP = est.enter_context(tc.tile_pool(name="pP", bufs=1))
        pDram = est.enter_context(tc.tile_pool(name="pDram", bufs=1, space="DRAM"))
        bcd = pDram.tile([2 * D_STATE, NCH, CH], BF16)

        # persistent state
        gT = pP.tile([128, NPT, SEQ], F32)        # silu(z)
        xc_own = pP.tile([128, NPT, SEQ], BF16)   # silu(conv(xi)), own half
        dtT = pP.tile([128, NPT, SEQ], BF16)
        uT = pP.tile([128, NPT, SEQ], BF16)
        sb_xT = pP.tile([128, 4, SEQ], F32R)
        xi_hist = pP.tile([128, NFT, 3], BF16)    # conv history per m-tile
        # weights (resident)
        sb_win = pP.tile([128, 4, D_INNER + HALF], F32R)
        sb_cb = pP.tile([128, NFT, 1], F32)
        sb_wxp = pP.tile([128, NFT, NXD], BF16)
        sb_wdt = pP.tile([DT_RANK, HALF], F32R)
        sb_dtb = pP.tile([128, NPT, 1], F32)
        sb_A = pP.tile([128, NPT, D_STATE], F32)
        sb_dD = pP.tile([128, NPT, 128], BF16)
        sb_id = pP.tile([128, 128], BF16)
        sb_wout = pP.tile([128, NPT, DIM], F32R)
        sb_cd = pP.tile([128, NFT, D_CONV, 128], BF16)  # diag(conv_w) tiles

        # input DMAs: x chunks on the (initially idle) Pool SWDGE queue,
        # weights on SP/Act (HWDGE engines)
        for c in range(NCH):
            nc.gpsimd.dma_start(sb_xT[:, :, c * CH:(c + 1) * CH],
                                xT[:, :, c * CH:(c + 1) * CH])
        nc.sync.dma_start(sb_win[:], w_in[:])
        nc.sync.dma_start(sb_cb[:], convb[:])
        nc.sync.dma_start(sb_cd[:], convd[:])
        nc.sync.dma_start(sb_wxp[:], w_xp[:])
        nc.scalar.dma_start(sb_wdt[:], w_dt[:])
        nc.scalar.dma_start(sb_dtb[:], dtb[:])
        nc.scalar.dma_start(sb_A[:], Acol[:])
        nc.scalar.dma_start(sb_dD[:], diagD[:])
        nc.scalar.dma_start(sb_id[:], ident[:])
        nc.scalar.dma_start(sb_wout[:], w_out[:])

        pXi = est.enter_context(tc.tile_pool(name="pXi", bufs=2))
        pSm = est.enter_context(tc.tile_pool(name="pSm", bufs=1))
        pBc = est.enter_context(tc.tile_pool(name="pBc", bufs=3))
        pCdA = est.enter_context(tc.tile_pool(name="pCdA", bufs=4))
        pCw = est.enter_context(tc.tile_pool(name="pCw", bufs=1))
        pChc = est.enter_context(tc.tile_pool(name="pChc", bufs=2))
        pG = est.enter_context(tc.tile_pool(name="pG", bufs=1))
        pHl = est.enter_context(tc.tile_pool(name="pHl", bufs=2))
        pOt = est.enter_context(tc.tile_pool(name="pOt", bufs=1))
        psP = est.enter_context(tc.tile_pool(name="psP", bufs=1, space="PSUM"))

        psx = {}   # x_proj accumulators per chunk (1 bank, cycled)
        sl = {}    # token slices
        for c in range(NCH):
            sl[c] = slice(c * CH, (c + 1) * CH)

        def emit_A_unit(c, m):
            """in_proj + conv + silu + x_proj-acc for (chunk c, unit m).
            Units 0..7 = xi channel tiles, 8..11 = z tiles."""
            ps = psP.tile([128, CH], F32, tag="mm", bufs=2)
            for k in range(4):
                nc.tensor.matmul(ps[:], sb_win[:, k, m * 128:(m + 1) * 128],
                                 sb_xT[:, k, sl[c]], start=(k == 0), stop=(k == 3))
            if m >= 8:
                nc.scalar.activation(gT[:, m - 8, sl[c]], ps[:], AF.Silu)
                return
            xi_c = pXi.tile([128, 3 + CH], BF16, tag="xi")
            if c == 0:
                nc.vector.memset(xi_c[:, 0:3], 0.0)
            else:
                nc.vector.tensor_copy(xi_c[:, 0:3], xi_hist[:, m, :])
            nc.scalar.activation(xi_c[:, 3:3 + CH], ps[:], AF.Copy)
            if c < NCH - 1:
                nc.vector.tensor_copy(xi_hist[:, m, :], xi_c[:, CH:CH + 3])
            ps2 = psP.tile([128, CH], F32, tag="mm", bufs=2)
            for k in range(D_CONV):
                nc.tensor.matmul(ps2[:], sb_cd[:, m, k, :], xi_c[:, k:k + CH],
                                 start=(k == 0), stop=(k == D_CONV - 1))
            if m < NPT:
                xco = xc_own[:, m, sl[c]]
            else:
                xco_t = pXi.tile([128, CH], BF16, tag="xco")
                xco = xco_t[:]
            nc.scalar.activation(xco, ps2[:], AF.Silu, bias=sb_cb[:, m, :])
            if m == 0:
                psx[c] = psP.tile([NXD, CH], F32, tag="xp", bufs=1, name="psxc")
            nc.tensor.matmul(psx[c][:], sb_wxp[:, m, :], xco,
                             start=(m == 0), stop=(m == NFT - 1))

        def emit_B(c):
            """x_proj unload, B/C rows to DRAM, dt softplus, u."""
            dbc = pSm.tile([DT_RANK, CH], F32R, tag="dbc")
            bcb = pSm.tile([2 * D_STATE, CH], BF16, tag="bcb")
            nc.scalar.activation(dbc[:], psx[c][0:DT_RANK, :], AF.Copy)
            nc.scalar.activation(bcb[:], psx[c][DT_RANK:NXD, :], AF.Copy)
            nc.sync.dma_start(bcd[:, c, :], bcb[:])
            for mt in range(NPT):
                ps3 = psP.tile([128, CH], F32, tag="aux", bufs=1)
                nc.tensor.matmul(ps3[:], sb_wdt[:, mt * 128:(mt + 1) * 128],
                                 dbc[:], start=True, stop=True)
                spe = pSm.tile([128, CH], F32, tag="spe")
                nc.scalar.activation(spe[:], ps3[:], AF.Exp, bias=sb_dtb[:, mt, :])
                nc.scalar.activation(dtT[:, mt, sl[c]], spe[:], AF.Ln, bias=1.0)
            nc.vector.tensor_mul(uT[:, :, sl[c]], dtT[:, :, sl[c]],
                                 xc_own[:, :, sl[c]])

        def emit_bcast(c, s):
            """One partition-step-0 DMA: B_s and C_s rows for chunk c."""
            bc2 = pBc.tile([128, 2, CH], BF16, tag="bc")
            row = bcd[s:s + 1, c, :]
            eng = nc.sync if (s % 2 == 0) else nc.gpsimd
            eng.dma_start(bc2[:], _bcast_rows(row, 2, D_STATE * NCH * CH))
            return bc2

        hl_prev = [None]

        def emit_C(c, weave=None):
            """Scan chunk c; weave(s) emits next chunk's A units between
            scan iterations to keep PE/Act busy."""
            yps = psP.tile([128, NPT, CH], F32, tag="yps", bufs=1)
            for pt in range(NPT):
                nc.tensor.matmul(yps[:, pt, :], sb_dD[:, pt, :],
                                 xc_own[:, pt, sl[c]],
                                 start=True, stop=False, skip_group_check=True)
            hl_cur = pHl.tile([128, NPT * D_STATE], F32, tag="hl")
            bcq = [emit_bcast(c, s) for s in range(3)]
            for s in range(D_STATE):
                if s + 3 < D_STATE:
                    bcq.append(emit_bcast(c, s + 3))
                bc2 = bcq[s]
                dAs = []
                for pt in range(NPT):
                    dA = pCdA.tile([128, CH], F32R, tag="dA")
                    nc.scalar.activation(dA[:], dtT[:, pt, sl[c]], AF.Exp,
                                         scale=sb_A[:, pt, s:s + 1])
                    dAs.append(dA)
                dBu = pCw.tile([128, NPT, CH], BF16, tag="dBu")
                nc.vector.tensor_mul(dBu[:], uT[:, :, sl[c]],
                                     _mid_bcast(bc2[:, 0, :], NPT))
                h = pCw.tile([128, NPT, CH], BF16, tag="h")
                for pt in range(NPT):
                    init = 0.0 if c == 0 else \
                        hl_prev[0][:, s * NPT + pt:s * NPT + pt + 1]
                    nc.vector.tensor_tensor_scan(
                        h[:, pt, :], dAs[pt][:], dBu[:, pt, :], init,
                        OP.mult, OP.add)
                if c < NCH - 1:
                    nc.vector.tensor_copy(
                        hl_cur[:, s * NPT:(s + 1) * NPT], h[:, :, CH - 1:CH])
                hc = pChc.tile([128, NPT, CH], BF16, tag="hc")
                nc.vector.tensor_mul(hc[:], h[:],
                                     _mid_bcast(bc2[:, 1, :], NPT))
                for pt in range(NPT):
                    nc.tensor.matmul(yps[:, pt, :], sb_id[:], hc[:, pt, :],
                                     start=False, stop=(s == D_STATE - 1),
                                     skip_group_check=True)
                if weave is not None:
                    weave(s)
            hl_prev[0] = hl_cur
            return yps

        def emit_gate_D(c, yps):
            y_gc = pG.tile([128, NPT, CH], F32R, tag="yg")
            nc.vector.tensor_mul(y_gc[:], yps[:, :, :], gT[:, :, sl[c]])
            for mt in range(NPT):
                ps4 = psP.tile([128, CH], F32, tag="aux", bufs=1)
                for k in range(NPT):
                    nc.tensor.matmul(ps4[:], sb_wout[:, k, mt * 128:(mt + 1) * 128],
                                     y_gc[:, k, :], start=(k == 0),
                                     stop=(k == NPT - 1))
                ot = pOt.tile([128, CH], F32, tag="ot")
                nc.scalar.activation(ot[:], ps4[:], AF.Copy)
                nc.sync.dma_start(oT[:, mt, sl[c]], ot[:])

        # prologue: chunk 0 A+B
        for m in range(12):
            emit_A_unit(0, m)
        emit_B(0)
        # pipelined chunks
        for c in range(NCH):
            if c + 1 < NCH:
                sched = {4: list(range(6)), 10: list(range(6, 12))}

                def weave(s, c=c, sched=sched):
                    for m in sched.get(s, []):
                        emit_A_unit(c + 1, m)
                yps = emit_C(c, weave)
                emit_B(c + 1)
            else:
                yps = emit_C(c, None)
            emit_gate_D(c, yps)

    nc.compile()
    _PROG_CACHE["nc"] = nc
    return nc


def _prep_core_inputs(x, params, direction, batch, half):
    in_w, conv_w, conv_b, xproj_w, dt_w, dt_b, A_log, D, out_w = params
    xb = x[batch]
    if direction == 1:
        xb = xb[::-1]
    xT = np.ascontiguousarray(xb.T)

    own = np.arange(half * HALF, (half + 1) * HALF)
    other = np.arange((1 - half) * HALF, (2 - half) * HALF)
    perm = np.concatenate([own, other])

    w_in = np.concatenate([in_w[perm], in_w[D_INNER + own]], axis=0).T  # [512, 1536]
    cw = conv_w[perm, 0, :]                                            # [1024, 4]
    convw_ = cw.reshape(NFT, 128, D_CONV).transpose(1, 0, 2)           # [128, NFT, 4]
    convd_ = np.zeros((128, NFT, D_CONV, 128), np.float32)
    jj = np.arange(128)
    for mft in range(NFT):
        for kk in range(D_CONV):
            convd_[jj, mft, kk, jj] = cw.reshape(NFT, 128, D_CONV)[mft, :, kk]
    ii = np.arange(128)
    convb_ = conv_b[perm].reshape(NFT, 128, 1).transpose(1, 0, 2)
    w_xp = xproj_w[:, perm].T.reshape(NFT, 128, -1).transpose(1, 0, 2)
    w_dt_ = np.ascontiguousarray(dt_w[own].T)
    dtb_ = dt_b[own].reshape(NPT, 128, 1).transpose(1, 0, 2)
    Acol_ = (-np.exp(A_log[own])).reshape(NPT, 128, D_STATE).transpose(1, 0, 2)
    dD = np.zeros((128, NPT, 128), np.float32)
    Dr = D[own].reshape(NPT, 128)
    for ptn in range(NPT):
        dD[ii, ptn, ii] = Dr[ptn]
    w_out_ = out_w[:, own].T.reshape(NPT, 128, DIM).transpose(1, 0, 2)

    def c32(a):
        return np.ascontiguousarray(a, dtype=np.float32)

    return {
        "xT": c32(xT.reshape(4, 128, SEQ).transpose(1, 0, 2)),
        "w_in": c32(w_in.reshape(4, 128, -1).transpose(1, 0, 2)),
        "convw": c32(convw_),
        "convd": convd_.astype(NPBF16),
        "convb": c32(convb_),
        "w_xp": np.ascontiguousarray(w_xp, dtype=NPBF16),
        "w_dt": c32(w_dt_),
        "dtb": c32(dtb_),
        "Acol": c32(Acol_),
        "diagD": np.ascontiguousarray(dD, dtype=NPBF16),
        "ident": np.eye(128, dtype=NPBF16),
        "w_out": c32(w_out_),
    }


def _run(nc, in_maps):
    if os.environ.get("BIMAMBA_SIM"):
        from concourse.bass_interp import CoreSim
        results = []
        n = int(os.environ.get("BIMAMBA_SIM_CORES", "8"))
        for m in in_maps[:n]:
            sim = CoreSim(nc)
            for k, v in m.items():
                sim.tensor(k)[:] = v
            sim.simulate()
            results.append({"oT": np.array(sim.tensor("oT"))})
        return results
    return run_bass_kernel_spmd(nc, in_maps, core_ids=list(range(8))).results


def _prep_all(inputs):
    x = np.asarray(inputs["x"], np.float32)
    names = ["in_w", "conv_w", "conv_b", "xproj_w", "dt_w", "dt_b", "A_log", "D", "out_w"]
    fp = tuple(np.asarray(inputs["f_" + n], np.float32) for n in names)
    rp = tuple(np.asarray(inputs["r_" + n], np.float32) for n in names)
    in_maps = []
    meta = []
    for d in (0, 1):
        for b in range(B_SZ):
            for h in (0, 1):
                in_maps.append(_prep_core_inputs(x, fp if d == 0 else rp, d, b, h))
                meta.append((d, b, h))
    return in_maps, meta


def kernel(**inputs):
    nc = _build_program()
    in_maps, meta = _prep_all(inputs)
    results = _run(nc, in_maps)

    acc = np.zeros((2, B_SZ, SEQ, DIM), np.float32)
    for (d, b, h), r in zip(meta, results):
        oTv = r["oT"]
        o = oTv.transpose(1, 0, 2).reshape(DIM, SEQ).T
        if d == 1:
            o = o[::-1]
        acc[d, b] += o
    out = 0.5 * (acc[0] + acc[1])
    return out.astype(np.float32)


# revision 11
# speedup vs baseline: 1.2321x; 1.0004x over previous
"""BiMamba Trainium2 kernel, chunk-pipelined, exp-table-only activations (v4).

Sharding: 8 cores = (direction f/r) x (batch 2) x (d_inner half 2), SPMD.
Host permutes channels so each core's own 512 scan channels are first;
xi/conv/x_proj cover all 1024 channels per core; out_proj partials summed
on host; reverse direction flipped on host.

v3 restructure: the sequence is processed in 4 chunks of 512 tokens and
the whole per-chunk pipeline (in_proj/conv -> x_proj -> dt/B/C -> scan ->
gate -> out_proj) is software-pipelined: chunk c+1's PE/Act work (in_proj,
conv, silu) is emitted woven between chunk c's scan iterations so the
in-order engine queues overlap phases.  Scans are chunked and chained via
a per-(s,pt) boundary-state tile (tensor_tensor_scan initial=).  The
selective-scan elementwise ops run as fused [128,4,512] 3D-AP ops (the
B/C broadcast operand uses a 0-stride middle dim) since DVE is the
bottleneck engine: scans + float muls are DVE-only on TRN2 (Pool's ISA
rejects the scan opcode and float tensor_tensor; Pool cannot touch PSUM).
B_s/C_s rows for one (chunk, s) are fetched by a single partition-step-0
DMA shared by all 4 channel tiles, alternating SP/Pool queues, prefetched
3 states ahead.
"""
import os
from contextlib import ExitStack

import numpy as np

import concourse.bacc as bacc
import concourse.bass as bass
import concourse.tile as tile
from concourse import mybir
from concourse.bass_utils import run_bass_kernel_spmd

F32 = mybir.dt.float32
BF16 = mybir.dt.bfloat16
F32R = mybir.dt.float32r
AF = mybir.ActivationFunctionType
OP = mybir.AluOpType
NPBF16 = mybir.dt.np(mybir.dt.bfloat16)

DIM = 512
D_STATE = 16
D_CONV = 4
D_INNER = 1024
DT_RANK = 32
B_SZ = 2
SEQ = 2048
HALF = 512
NPT = HALF // 128     # 4 own-channel partition tiles
NFT = D_INNER // 128  # 8 full-channel partition tiles
NXD = DT_RANK + 2 * D_STATE  # 64
CH = 512              # token chunk
NCH = SEQ // CH       # 4 chunks

_PROG_CACHE = {}


def _bcast_rows(src_row, n_rows, row_stride, n_part=128):
    """Partition-step-0 AP: every partition reads the same n_rows rows."""
    return bass.AP(
        tensor=src_row.tensor, offset=src_row.offset,
        ap=[[0, n_part], [row_stride, n_rows]] + list(src_row.ap[1:]))


def _mid_bcast(sl2d, n_mid):
    """[128, N] slice -> [128, n_mid, N] AP with 0-stride middle dim."""
    return bass.AP(
        tensor=sl2d.tensor, offset=sl2d.offset,
        ap=[list(sl2d.ap[0]), [0, n_mid], list(sl2d.ap[-1])])


def _build_program():
    if "nc" in _PROG_CACHE:
        return _PROG_CACHE["nc"]
    nc = bacc.Bacc("TRN2", target_bir_lowering=False, debug=False)

    xT = nc.dram_tensor("xT", [128, 4, SEQ], F32R, kind="ExternalInput")
    w_in = nc.dram_tensor("w_in", [128, 4, D_INNER + HALF], F32R, kind="ExternalInput")
    convw = nc.dram_tensor("convw", [128, NFT, D_CONV], F32, kind="ExternalInput")
    convd = nc.dram_tensor("convd", [128, NFT, D_CONV, 128], BF16, kind="ExternalInput")
    convbd = nc.dram_tensor("convbd", [128, NFT, 128], BF16, kind="ExternalInput")
    w_xp = nc.dram_tensor("w_xp", [128, NFT, NXD], BF16, kind="ExternalInput")
    w_dt = nc.dram_tensor("w_dt", [DT_RANK, HALF], F32R, kind="ExternalInput")
    dtb = nc.dram_tensor("dtb", [128, NPT, 1], F32, kind="ExternalInput")
    Acol = nc.dram_tensor("Acol", [128, NPT, D_STATE], F32, kind="ExternalInput")
    diagD = nc.dram_tensor("diagD", [128, NPT, 128], BF16, kind="ExternalInput")
    ident = nc.dram_tensor("ident", [128, 128], BF16, kind="ExternalInput")
    w_out = nc.dram_tensor("w_out", [128, NPT, DIM], F32R, kind="ExternalInput")
    oT = nc.dram_tensor("oT", [128, 4, SEQ], F32, kind="ExternalOutput")

    loop_n = int(os.environ.get("BIMAMBA_LOOP", "0"))
    with tile.TileContext(nc) as tc, ExitStack() as est:
        if loop_n > 1:
            est.enter_context(tc.For_i(0, loop_n, 1))
        pP = est.enter_context(tc.tile_pool(name="pP", bufs=1))
        pDram = est.enter_context(tc.tile_pool(name="pDram", bufs=1, space="DRAM"))
        bcd = pDram.tile([2 * D_STATE, NCH, CH], BF16)

        # persistent state
        gT = pP.tile([128, NPT, SEQ], BF16)       # 2*silu(z) (w_out prescaled)
        xc_own = pP.tile([128, NPT, SEQ], BF16)   # silu(conv(xi)), own half
        dtT = pP.tile([128, NPT, SEQ], BF16)
        uT = pP.tile([128, NPT, SEQ], BF16)
        sb_xT = pP.tile([128, 4, SEQ], F32R)
        xi_hist = pP.tile([128, NFT, 3], BF16)    # conv history per m-tile
        # weights (resident)
        sb_win = pP.tile([128, 4, D_INNER + HALF], F32R)
        sb_cbd = pP.tile([128, NFT, 128], BF16)
        onesb = pP.tile([128, CH], BF16)
        sb_wxp = pP.tile([128, NFT, NXD], BF16)
        sb_wdt = pP.tile([DT_RANK, HALF], F32R)
        sb_dtb = pP.tile([128, NPT, 1], F32)
        sb_A = pP.tile([128, NPT, D_STATE], F32)
        sb_dD = pP.tile([128, NPT, 128], BF16)
        sb_id = pP.tile([128, 128], BF16)
        sb_wout = pP.tile([128, NPT, DIM], F32R)
        sb_cd = pP.tile([128, NFT, D_CONV, 128], BF16)  # diag(conv_w) tiles

        # input DMAs: x chunks on the (initially idle) Pool SWDGE queue,
        # weights on SP/Act (HWDGE engines)
        for c in range(NCH):
            nc.gpsimd.dma_start(sb_xT[:, :, c * CH:(c + 1) * CH],
                                xT[:, :, c * CH:(c + 1) * CH])
        nc.sync.dma_start(sb_win[:], w_in[:])
        nc.sync.dma_start(sb_cbd[:], convbd[:])
        nc.sync.dma_start(sb_cd[:], convd[:])
        nc.sync.dma_start(sb_wxp[:], w_xp[:])
        nc.scalar.dma_start(sb_wdt[:], w_dt[:])
        nc.scalar.dma_start(sb_dtb[:], dtb[:])
        nc.scalar.dma_start(sb_A[:], Acol[:])
        nc.scalar.dma_start(sb_dD[:], diagD[:])
        nc.scalar.dma_start(sb_id[:], ident[:])
        nc.scalar.dma_start(sb_wout[:], w_out[:])
        nc.vector.memset(onesb[:], 1.0)

        pXi = est.enter_context(tc.tile_pool(name="pXi", bufs=2))
        pSm = est.enter_context(tc.tile_pool(name="pSm", bufs=1))
        pBc = est.enter_context(tc.tile_pool(name="pBc", bufs=4))
        pCdA = est.enter_context(tc.tile_pool(name="pCdA", bufs=6))
        pCw = est.enter_context(tc.tile_pool(name="pCw", bufs=1))
        pChc = est.enter_context(tc.tile_pool(name="pChc", bufs=2))
        pG = est.enter_context(tc.tile_pool(name="pG", bufs=1))
        pHl = est.enter_context(tc.tile_pool(name="pHl", bufs=2))
        pOt = est.enter_context(tc.tile_pool(name="pOt", bufs=1))
        psP = est.enter_context(tc.tile_pool(name="psP", bufs=1, space="PSUM"))

        psx = {}   # x_proj accumulators per chunk (1 bank, cycled)
        sl = {}    # token slices
        for c in range(NCH):
            sl[c] = slice(c * CH, (c + 1) * CH)

        def emit_A_unit(c, m):
            """in_proj + conv + silu + x_proj-acc for (chunk c, unit m).
            Units 0..7 = xi channel tiles, 8..11 = z tiles."""
            ps = psP.tile([128, CH], F32, tag="mm", bufs=2)
            for k in range(4):
                nc.tensor.matmul(ps[:], sb_win[:, k, m * 128:(m + 1) * 128],
                                 sb_xT[:, k, sl[c]], start=(k == 0), stop=(k == 3))
            if m >= 8:
                # 2*silu(z) = z*(1+tanh(z/2)); the 1/2 is folded into w_out
                thz = pXi.tile([128, CH], BF16, tag="thz")
                nc.scalar.activation(thz[:], ps[:], AF.Tanh, scale=0.5)
                nc.vector.scalar_tensor_tensor(
                    gT[:, m - 8, sl[c]], thz[:], 1.0, ps[:], OP.add, OP.mult)
                return
            xi_c = pXi.tile([128, 3 + CH], BF16, tag="xi")
            if c == 0:
                nc.vector.memset(xi_c[:, 0:3], 0.0)
            else:
                nc.vector.tensor_copy(xi_c[:, 0:3], xi_hist[:, m, :])
            nc.scalar.activation(xi_c[:, 3:3 + CH], ps[:], AF.Copy)
            if c < NCH - 1:
                nc.vector.tensor_copy(xi_hist[:, m, :], xi_c[:, CH:CH + 3])
            ps2 = psP.tile([128, CH], F32, tag="mm", bufs=2)
            nc.tensor.matmul(ps2[:], sb_cbd[:, m, :], onesb[:],
                             start=True, stop=False)
            for k in range(D_CONV):
                nc.tensor.matmul(ps2[:], sb_cd[:, m, k, :], xi_c[:, k:k + CH],
                                 start=False, stop=(k == D_CONV - 1))
            if m < NPT:
                xco = xc_own[:, m, sl[c]]
            else:
                xco_t = pXi.tile([128, CH], BF16, tag="xco")
                xco = xco_t[:]
            # 2*silu(v) = v*(1+tanh(v/2)), v = conv + bias (bias via the
            # extra diag matmul); the 2x is compensated in w_xp/w_out/u
            thc = pXi.tile([128, CH], BF16, tag="thc")
            nc.scalar.activation(thc[:], ps2[:], AF.Tanh, scale=0.5)
            nc.vector.scalar_tensor_tensor(xco, thc[:], 1.0, ps2[:],
                                           OP.add, OP.mult)
            if m == 0:
                psx[c] = psP.tile([NXD, CH], F32, tag="xp", bufs=1, name="psxc")
            nc.tensor.matmul(psx[c][:], sb_wxp[:, m, :], xco,
                             start=(m == 0), stop=(m == NFT - 1))

        def emit_B(c):
            """x_proj unload, B/C rows to DRAM, dt softplus, u."""
            dbc = pSm.tile([DT_RANK, CH], F32R, tag="dbc")
            bcb = pSm.tile([2 * D_STATE, CH], BF16, tag="bcb")
            nc.scalar.activation(dbc[:], psx[c][0:DT_RANK, :], AF.Copy)
            nc.scalar.activation(bcb[:], psx[c][DT_RANK:NXD, :], AF.Copy)
            nc.sync.dma_start(bcd[:, c, :], bcb[:])
            for mt in range(NPT):
                ps3 = psP.tile([128, CH], F32, tag="aux", bufs=1)
                nc.tensor.matmul(ps3[:], sb_wdt[:, mt * 128:(mt + 1) * 128],
                                 dbc[:], start=True, stop=True)
                spe = pSm.tile([128, CH], F32, tag="spe")
                nc.scalar.activation(spe[:], ps3[:], AF.Exp, bias=sb_dtb[:, mt, :])
                nc.scalar.activation(dtT[:, mt, sl[c]], spe[:], AF.Ln, bias=1.0)
            nc.vector.tensor_mul(uT[:, :, sl[c]], dtT[:, :, sl[c]],
                                 xc_own[:, :, sl[c]])

        def emit_bcast(c, s):
            """One partition-step-0 DMA: B_s and C_s rows for chunk c."""
            bc2 = pBc.tile([128, 2, CH], BF16, tag="bc")
            row = bcd[s:s + 1, c, :]
            eng = nc.sync if (s % 2 == 0) else nc.gpsimd
            eng.dma_start(bc2[:], _bcast_rows(row, 2, D_STATE * NCH * CH))
            return bc2

        hl_prev = [None]

        def emit_C(c, weave=None):
            """Scan chunk c; weave(s) emits next chunk's A units between
            scan iterations to keep PE/Act busy."""
            yps = psP.tile([128, NPT, CH], F32, tag="yps", bufs=1)
            for pt in range(NPT):
                nc.tensor.matmul(yps[:, pt, :], sb_dD[:, pt, :],
                                 xc_own[:, pt, sl[c]],
                                 start=True, stop=False, skip_group_check=True)
            hl_cur = pHl.tile([128, NPT * D_STATE], F32, tag="hl")
            bcq = [emit_bcast(c, s) for s in range(3)]
            for s in range(D_STATE):
                if s + 3 < D_STATE:
                    bcq.append(emit_bcast(c, s + 3))
                bc2 = bcq[s]
                dAs = []
                for pt in range(NPT):
                    dA = pCdA.tile([128, CH], F32R, tag="dA")
                    nc.scalar.activation(dA[:], dtT[:, pt, sl[c]], AF.Exp,
                                         scale=sb_A[:, pt, s:s + 1])
                    dAs.append(dA)
                dBu = pCw.tile([128, NPT, CH], BF16, tag="dBu")
                nc.vector.tensor_mul(dBu[:], uT[:, :, sl[c]],
                                     _mid_bcast(bc2[:, 0, :], NPT))
                h = pCw.tile([128, NPT, CH], BF16, tag="h")
                for pt in range(NPT):
                    init = 0.0 if c == 0 else \
                        hl_prev[0][:, s * NPT + pt:s * NPT + pt + 1]
                    nc.vector.tensor_tensor_scan(
                        h[:, pt, :], dAs[pt][:], dBu[:, pt, :], init,
                        OP.mult, OP.add)
                if c < NCH - 1:
                    nc.vector.tensor_copy(
                        hl_cur[:, s * NPT:(s + 1) * NPT], h[:, :, CH - 1:CH])
                hc = pChc.tile([128, NPT, CH], BF16, tag="hc")
                nc.vector.tensor_mul(hc[:], h[:],
                                     _mid_bcast(bc2[:, 1, :], NPT))
                for pt in range(NPT):
                    nc.tensor.matmul(yps[:, pt, :], sb_id[:], hc[:, pt, :],
                                     start=False, stop=(s == D_STATE - 1),
                                     skip_group_check=True)
                if weave is not None:
                    weave(s)
            hl_prev[0] = hl_cur
            return yps

        def emit_gate_D(c, yps):
            y_gc = pG.tile([128, NPT, CH], F32R, tag="yg")
            nc.vector.tensor_mul(y_gc[:], yps[:, :, :], gT[:, :, sl[c]])
            for mt in range(NPT):
                ps4 = psP.tile([128, CH], F32, tag="aux", bufs=1)
                for k in range(NPT):
                    nc.tensor.matmul(ps4[:], sb_wout[:, k, mt * 128:(mt + 1) * 128],
                                     y_gc[:, k, :], start=(k == 0),
                                     stop=(k == NPT - 1))
                ot = pOt.tile([128, CH], F32, tag="ot")
                nc.scalar.activation(ot[:], ps4[:], AF.Copy)
                nc.sync.dma_start(oT[:, mt, sl[c]], ot[:])

        # prologue: chunk 0 A+B
        for m in range(12):
            emit_A_unit(0, m)
        emit_B(0)
        # pipelined chunks
        for c in range(NCH):
            if c + 1 < NCH:
                sched = {4: list(range(6)), 10: list(range(6, 12))}

                def weave(s, c=c, sched=sched):
                    for m in sched.get(s, []):
                        emit_A_unit(c + 1, m)
                yps = emit_C(c, weave)
                emit_B(c + 1)
            else:
                yps = emit_C(c, None)
            emit_gate_D(c, yps)

    nc.compile()
    _PROG_CACHE["nc"] = nc
    return nc


def _prep_core_inputs(x, params, direction, batch, half):
    in_w, conv_w, conv_b, xproj_w, dt_w, dt_b, A_log, D, out_w = params
    xb = x[batch]
    if direction == 1:
        xb = xb[::-1]
    xT = np.ascontiguousarray(xb.T)

    own = np.arange(half * HALF, (half + 1) * HALF)
    other = np.arange((1 - half) * HALF, (2 - half) * HALF)
    perm = np.concatenate([own, other])

    w_in = np.concatenate([in_w[perm], in_w[D_INNER + own]], axis=0).T  # [512, 1536]
    cw = conv_w[perm, 0, :]                                            # [1024, 4]
    convw_ = cw.reshape(NFT, 128, D_CONV).transpose(1, 0, 2)           # [128, NFT, 4]
    convd_ = np.zeros((128, NFT, D_CONV, 128), np.float32)
    convbd_ = np.zeros((128, NFT, 128), np.float32)
    jj = np.arange(128)
    cbm = conv_b[perm].reshape(NFT, 128)
    for mft in range(NFT):
        convbd_[jj, mft, jj] = cbm[mft]
        for kk in range(D_CONV):
            convd_[jj, mft, kk, jj] = cw.reshape(NFT, 128, D_CONV)[mft, :, kk]
    ii = np.arange(128)
    convb_ = conv_b[perm].reshape(NFT, 128, 1).transpose(1, 0, 2)
    w_xp = xproj_w[:, perm].T.reshape(NFT, 128, -1).transpose(1, 0, 2)
    w_dt_ = np.ascontiguousarray(dt_w[own].T)
    dtb_ = dt_b[own].reshape(NPT, 128, 1).transpose(1, 0, 2)
    Acol_ = (-np.exp(A_log[own])).reshape(NPT, 128, D_STATE).transpose(1, 0, 2)
    dD = np.zeros((128, NPT, 128), np.float32)
    Dr = D[own].reshape(NPT, 128)
    for ptn in range(NPT):
        dD[ii, ptn, ii] = Dr[ptn]
    w_out_ = out_w[:, own].T.reshape(NPT, 128, DIM).transpose(1, 0, 2)

    def c32(a):
        return np.ascontiguousarray(a, dtype=np.float32)

    return {
        "xT": c32(xT.reshape(4, 128, SEQ).transpose(1, 0, 2)),
        "w_in": c32(w_in.reshape(4, 128, -1).transpose(1, 0, 2)),
        "convw": c32(convw_),
        "convd": convd_.astype(NPBF16),
        "convbd": convbd_.astype(NPBF16),
        "w_xp": np.ascontiguousarray(0.5 * w_xp, dtype=NPBF16),
        "w_dt": c32(w_dt_),
        "dtb": c32(dtb_),
        "Acol": c32(Acol_),
        "diagD": np.ascontiguousarray(dD, dtype=NPBF16),
        "ident": np.eye(128, dtype=NPBF16),
        "w_out": c32(0.25 * w_out_),
    }


def _run(nc, in_maps):
    if os.environ.get("BIMAMBA_SIM"):
        from concourse.bass_interp import CoreSim
        results = []
        n = int(os.environ.get("BIMAMBA_SIM_CORES", "8"))
        for m in in_maps[:n]:
            sim = CoreSim(nc)
            for k, v in m.items():
                sim.tensor(k)[:] = v
            sim.simulate()
            results.append({"oT": np.array(sim.tensor("oT"))})
        return results
    return run_bass_kernel_spmd(nc, in_maps, core_ids=list(range(8))).results


def _prep_all(inputs):
    x = np.asarray(inputs["x"], np.float32)
    names = ["in_w", "conv_w", "conv_b", "xproj_w", "dt_w", "dt_b", "A_log", "D", "out_w"]
    fp = tuple(np.asarray(inputs["f_" + n], np.float32) for n in names)
    rp = tuple(np.asarray(inputs["r_" + n], np.float32) for n in names)
    in_maps = []
    meta = []
    for d in (0, 1):
        for b in range(B_SZ):
            for h in (0, 1):
                in_maps.append(_prep_core_inputs(x, fp if d == 0 else rp, d, b, h))
                meta.append((d, b, h))
    return in_maps, meta


def kernel(**inputs):
    nc = _build_program()
    in_maps, meta = _prep_all(inputs)
    results = _run(nc, in_maps)

    acc = np.zeros((2, B_SZ, SEQ, DIM), np.float32)
    for (d, b, h), r in zip(meta, results):
        oTv = r["oT"]
        o = oTv.transpose(1, 0, 2).reshape(DIM, SEQ).T
        if d == 1:
            o = o[::-1]
        acc[d, b] += o
    out = 0.5 * (acc[0] + acc[1])
    return out.astype(np.float32)
